# revision 1
# baseline (speedup 1.0000x reference)
# Trainium2 Bass kernel for nn_DeepGCN (8-layer GENConv DeepGCN, N=40000, E=400000).
#
# v2 strategy (8 NeuronCores, SPMD single program):
#   - Nodes partitioned: core c owns 5000 contiguous schedule-order nodes as
#     20 slots x 250 nodes (LPT balance of in-degree via global permutation).
#   - Edges partitioned by dst slot; within a slot edges are sorted by
#     (permuted) src id, packed into uniform NCH_S chunks of 128, gathered by
#     dma_gather calls of <= 8 chunks with static per-call-index base offsets
#     (src-sorted quantile ranges always fit the 32768 int16 index window).
#   - Messages in bf16: m = relu(g + ea*ew), p = exp(t(m+eps)), q = (m+eps)p;
#     one-hot S [128e, 256] built by is_equal in bf16; per-chunk bf16 matmul
#     accumulates [q|p]^T @ S into a per-slot PSUM tile (256 wide).
#   - Node phase: bf16 MLP matmuls, BatchNorm via free-dim accum + 1 KB
#     AllReduce, residual in fp32, pre-norm BN + LeakyReLU, PE transpose to
#     node-major, AllGather (Shared output) of the next-layer feature table.
import os
import numpy as np

import sys
for _p in ("/opt/trn_rl_repo",):
    if _p not in sys.path:
        sys.path.insert(0, _p)

import ml_dtypes
import concourse.bass as bass
import concourse.bacc as bacc
import concourse.tile as tile
import concourse.mybir as mybir
from concourse import bass_utils

N, E, C_IN, C, CH, L = 40000, 400000, 16, 64, 128, 8
EPS, BN_EPS, SLOPE = 1e-7, 1e-5, 0.01
NC = 8
NPC = N // NC            # 5000 nodes per core
SLOTS = 20               # slots per core
SN = NPC // SLOTS        # 250 nodes per slot
SW = 256                 # PSUM width per slot
CALL_CH = 8              # max chunks per dma_gather call (1024 idx)
IDXW = 32768             # int16 index window
F32 = mybir.dt.float32
BF16 = mybir.dt.bfloat16
I16 = mybir.dt.int16
BF = ml_dtypes.bfloat16


def _shim_ntff_hook():
    """Register the axon NTFF profile hook (image lacks antenv.axon_hooks)."""
    import types
    if "antenv.axon_hooks" in sys.modules:
        return
    mod = types.ModuleType("antenv.axon_hooks")

    def set_hook(h):
        mod._hook = h

    def get_hook():
        return getattr(mod, "_hook", None)

    mod.set_axon_ntff_profile_hook = set_hook
    mod.get_axon_ntff_profile_hook = get_hook
    sys.modules["antenv.axon_hooks"] = mod
    try:
        if "/root/.axon_site" not in sys.path:
            sys.path.insert(0, "/root/.axon_site")
        from trn_agent_boot.trn_boot import _ntff_profile_via_ctypes
        set_hook(_ntff_profile_via_ctypes("/opt/axon/libaxon_pjrt.so"))
    except Exception:
        pass


def _wrap_idx(ix):
    """int array [L] (L%16==0) -> [128, L/16] int16 wrapped + 8x replicated."""
    Lx = len(ix)
    arr = ix.reshape(Lx // 16, 16).T.astype(np.int16)
    return np.tile(arr, (8, 1))


def _host_prep(inputs):
    src = np.asarray(inputs["edge_index"][0], dtype=np.int64)
    dst = np.asarray(inputs["edge_index"][1], dtype=np.int64)
    ea = np.asarray(inputs["edge_attr"][:, 0], dtype=np.float32)

    deg = np.bincount(dst, minlength=N)

    # --- per-core slot assignment (balance in-degree across slots) ---
    import heapq
    new_id = np.empty(N, dtype=np.int64)
    slot_of = np.empty(N, dtype=np.int64)
    pos_of = np.empty(N, dtype=np.int64)
    for c in range(NC):
        nodes = np.arange(c * NPC, (c + 1) * NPC)
        order = nodes[np.argsort(-deg[nodes], kind="stable")]
        heap = [(0, s) for s in range(SLOTS)]
        heapq.heapify(heap)
        counts = np.zeros(SLOTS, dtype=np.int64)
        for nd in order:
            while True:
                load, s = heapq.heappop(heap)
                if counts[s] < SN:
                    break
            slot_of[nd] = s
            pos_of[nd] = counts[s]
            counts[s] += 1
            if counts[s] < SN:
                heapq.heappush(heap, (load + deg[nd], s))
        new_id[nodes] = c * NPC + slot_of[nodes] * SN + pos_of[nodes]

    inv_perm = np.argsort(new_id)

    psrc = new_id[src]
    pdst = new_id[dst]

    # --- per (core, slot) edge lists, src-sorted, uniform chunk count ---
    order = np.argsort(pdst, kind="stable")
    psrc_s, pdst_s, ea_s = psrc[order], pdst[order], ea[order]
    starts = np.searchsorted(pdst_s, np.arange(0, N + 1, SN))

    slot_edges = []   # (core, slot) -> (src_sorted, dl, ea)
    max_cnt = 0
    for g in range(NC * SLOTS):
        lo_, hi_ = starts[g], starts[g + 1]
        e_src = psrc_s[lo_:hi_]
        e_dl = (pdst_s[lo_:hi_] - g * SN).astype(np.float32)
        e_ea = ea_s[lo_:hi_]
        so = np.argsort(e_src, kind="stable")
        slot_edges.append((e_src[so], e_dl[so], e_ea[so]))
        max_cnt = max(max_cnt, len(e_src))

    NCH_S = (max_cnt + 127) // 128       # uniform chunks per slot
    CPL = SLOTS * NCH_S                  # chunks per layer per core
    # fixed call split (chunk ranges) shared by all slots/cores
    splits = []
    k = 0
    while k < NCH_S:
        take = min(CALL_CH, NCH_S - k)
        splits.append((k, take))
        k += take

    # per-call-index static base offsets (over all cores+slots)
    bases = []
    for (c0, nch) in splits:
        mn, mx = N, 0
        for g in range(NC * SLOTS):
            e_src, e_dl, _ = slot_edges[g]
            seg = e_src[c0 * 128:(c0 + nch) * 128]
            if len(seg):
                mn = min(mn, int(seg.min()))
                mx = max(mx, int(seg.max()))
        base = max(0, mx - (IDXW - 1))
        assert base <= mn, (
            f"call split {c0}:{c0+nch} src range [{mn},{mx}] exceeds int16 "
            f"window; need adaptive bases")
        bases.append(base)

    # --- build per-core streams ---
    per_core = []
    for c in range(NC):
        idx_cols = []
        ea_cols = np.zeros((128, CPL), np.float32)
        dl_cols = np.full((128, CPL), -1.0, np.float32)
        for s in range(SLOTS):
            e_src, e_dl, e_ea = slot_edges[c * SLOTS + s]
            ne = len(e_src)
            npad = NCH_S * 128 - ne
            dlp = np.concatenate([e_dl, np.full(npad, -1.0, np.float32)])
            eap = np.concatenate([e_ea, np.zeros(npad, np.float32)])
            for kk in range(NCH_S):
                ch = s * NCH_S + kk
                ea_cols[:, ch] = eap[kk * 128:(kk + 1) * 128]
                dl_cols[:, ch] = dlp[kk * 128:(kk + 1) * 128]
            for j, (c0, nch) in enumerate(splits):
                seg = e_src[c0 * 128:(c0 + nch) * 128]
                rel = (seg - bases[j]).astype(np.int64)
                assert len(rel) == 0 or (rel.min() >= 0 and rel.max() < IDXW)
                segp = np.concatenate(
                    [rel, np.zeros(nch * 128 - len(rel), np.int64)])
                idx_cols.append(_wrap_idx(segp))
        idx_all = np.concatenate(idx_cols, axis=1)
        xg = np.asarray(inputs["x"])[inv_perm[c * NPC:(c + 1) * NPC]]
        per_core.append({
            "idx_all": idx_all,
            "ea_all": ea_cols,
            "dl_all": dl_cols.astype(BF),
            "xT": np.ascontiguousarray(xg.T.astype(np.float32)),
        })

    shared = {}
    shared["iota"] = np.tile(np.arange(SW, dtype=np.float32),
                             (128, 1)).astype(BF)
    sh = np.zeros((128, 64), np.float32)
    for j in range(64):
        sh[64 + j, j] = 1.0
    shared["shift"] = sh
    shared["identf"] = np.eye(64, dtype=np.float32)
    ew = np.concatenate([np.asarray(inputs["e0_w"]),
                         np.asarray(inputs["le_w"]).reshape(L - 1, C)], 0)
    ew_rep = np.repeat(ew[:, None, :], 128, axis=1).astype(np.float32)
    shared["ew_rep"] = np.ascontiguousarray(
        ew_rep.transpose(1, 0, 2).reshape(128, L * C))
    w1_all = np.concatenate([np.asarray(inputs["m0_w1"])[None],
                             np.asarray(inputs["lw1"])], 0).astype(np.float32)
    shared["w1"] = np.ascontiguousarray(
        w1_all.transpose(1, 0, 2).reshape(C, L * CH))
    w2_all = np.concatenate([np.asarray(inputs["m0_w2"])[None],
                             np.asarray(inputs["lw2"])], 0).astype(np.float32)
    shared["w2"] = np.ascontiguousarray(
        w2_all.transpose(1, 0, 2).reshape(CH, L * C))
    shared["lin0_w"] = np.asarray(inputs["lin0_w"]).astype(np.float32)
    eb = np.concatenate([np.asarray(inputs["e0_b"])[None],
                         np.asarray(inputs["le_b"])], 0)
    g1 = np.concatenate([np.asarray(inputs["m0_g1"])[None],
                         np.asarray(inputs["lg1"])], 0)
    be1 = np.concatenate([np.asarray(inputs["m0_be1"])[None],
                          np.asarray(inputs["lbe1"])], 0)
    b2 = np.concatenate([np.asarray(inputs["m0_b2"])[None],
                         np.asarray(inputs["lb2"])], 0)
    png = np.concatenate([np.asarray(inputs["lng"]),
                          np.asarray(inputs["norm0_g"])[None]], 0)
    pnb = np.concatenate([np.asarray(inputs["lnb"]),
                          np.asarray(inputs["norm0_b"])[None]], 0)
    ncol = L * 6 + 1
    colpack = np.zeros((128, ncol), np.float32)
    for l in range(L):
        colpack[:C, 6 * l + 0] = eb[l]
        colpack[:CH, 6 * l + 1] = g1[l]
        colpack[:CH, 6 * l + 2] = be1[l]
        colpack[:C, 6 * l + 3] = b2[l]
        colpack[:C, 6 * l + 4] = png[l]
        colpack[:C, 6 * l + 5] = pnb[l]
    colpack[:C, L * 6] = np.asarray(inputs["lin0_b"])
    shared["colpack"] = colpack
    tvals = [float(np.asarray(inputs["t0"])[0])] + \
        [float(v) for v in np.asarray(inputs["lt"])]

    meta = dict(NCH_S=NCH_S, CPL=CPL, splits=splits, bases=bases,
                tvals=tvals, inv_perm=inv_perm, new_id=new_id)
    return per_core, shared, meta


def _build_program(shared, meta):
    NCH_S, CPL = meta["NCH_S"], meta["CPL"]
    splits, bases = meta["splits"], meta["bases"]
    tvals = meta["tvals"]
    IDX_COLS = SLOTS * NCH_S * 8          # int16 cols (idx per slot wrapped)
    NCOL = L * 6 + 1
    MT = [(i * 512, min(512, NPC - i * 512)) for i in range((NPC + 511) // 512)]
    TT = [(i * 128, min(128, NPC - i * 128)) for i in range((NPC + 127) // 128)]

    nc = bacc.Bacc("TRN2", target_bir_lowering=False, debug=False,
                   num_devices=NC)
    d_idx = nc.dram_tensor("idx_all", [128, IDX_COLS], I16, kind="ExternalInput")
    d_ea = nc.dram_tensor("ea_all", [128, CPL], F32, kind="ExternalInput")
    d_dl = nc.dram_tensor("dl_all", [128, CPL], BF16, kind="ExternalInput")
    d_xT = nc.dram_tensor("xT", [C_IN, NPC], F32, kind="ExternalInput")
    d_iota = nc.dram_tensor("iota", [128, SW], BF16, kind="ExternalInput")
    d_shift = nc.dram_tensor("shift", [128, 64], F32, kind="ExternalInput")
    d_identf = nc.dram_tensor("identf", [64, 64], F32, kind="ExternalInput")
    d_ewr = nc.dram_tensor("ew_rep", [128, L * C], F32, kind="ExternalInput")
    d_w1 = nc.dram_tensor("w1", [C, L * CH], F32, kind="ExternalInput")
    d_w2 = nc.dram_tensor("w2", [CH, L * C], F32, kind="ExternalInput")
    d_l0w = nc.dram_tensor("lin0_w", [C_IN, C], F32, kind="ExternalInput")
    d_col = nc.dram_tensor("colpack", [128, NCOL], F32, kind="ExternalInput")
    d_out = nc.dram_tensor("out", [NPC, C], F32, kind="ExternalOutput")

    with tile.TileContext(nc) as tc:
        with (
            tc.tile_pool(name="const", bufs=1) as cp,
            tc.tile_pool(name="stage", bufs=3) as stp,
            tc.tile_pool(name="qp", bufs=3) as qpp,
            tc.tile_pool(name="spool", bufs=6) as sp,
            tc.tile_pool(name="node", bufs=1) as np_,
            tc.tile_pool(name="small", bufs=3) as smp,
            tc.tile_pool(name="pagg", bufs=2, space="PSUM") as pagg,
            tc.tile_pool(name="pmlp", bufs=2, space="PSUM") as pmlp,
            tc.tile_pool(name="paux", bufs=2, space="PSUM") as paux,
            tc.tile_pool(name="dram", bufs=1, space="DRAM") as dp,
        ):
            # ---------- resident constants ----------
            idx_all = cp.tile([128, IDX_COLS], I16)
            ea_all = cp.tile([128, CPL], F32)
            dl_all = cp.tile([128, CPL], BF16)
            iota = cp.tile([128, SW], BF16)
            shift = cp.tile([128, 64], F32)
            identf = cp.tile([64, 64], F32)
            ewr = cp.tile([128, L * C], F32)
            w1 = cp.tile([C, L * CH], F32)
            w2 = cp.tile([CH, L * C], F32)
            l0w = cp.tile([C_IN, C], F32)
            colp = cp.tile([128, NCOL], F32)
            xT = cp.tile([C_IN, NPC], F32)
            for t, d in [(idx_all, d_idx), (ea_all, d_ea), (dl_all, d_dl),
                         (iota, d_iota), (shift, d_shift),
                         (identf, d_identf),
                         (ewr, d_ewr), (w1, d_w1), (w2, d_w2), (l0w, d_l0w),
                         (colp, d_col), (xT, d_xT)]:
                nc.sync.dma_start(t[:], d[:])

            # ---------- persistent node buffers ----------
            z_work = np_.tile([64, NPC], F32)     # conv input (clean)
            agg_work = np_.tile([64, NPC], F32)  # genconv out (mlp input)
            h_work = np_.tile([64, NPC], F32)     # residual stream
            h1_work = np_.tile([128, NPC], F32)  # mlp hidden
            zt_work = np_.tile([64, NPC], F32)   # z + eb (table staging)
            dump = np_.tile([128, 512], F32)     # Square dump
            stats1 = np_.tile([128, 32], F32)
            stats2 = np_.tile([64, 32], F32)
            bnc1 = np_.tile([128, 8], F32)
            bnc2 = np_.tile([64, 8], F32)
            epsb = np_.tile([128, 8], F32)

            nc.vector.memset(epsb[:, 0:1], BN_EPS)

            # ---------- DRAM tiles ----------
            tabs = [dp.tile([N, C], F32, name=f"tab{l}", addr_space="Shared")
                    for l in range(L)]
            agb = [dp.tile([NPC, C], F32, name="agbA"),
                   dp.tile([NPC, C], F32, name="agbB")]
            arb1i = dp.tile([128, 2], F32, name="arb1i")
            arb1o = dp.tile([128, 2], F32, name="arb1o")
            arb2i = dp.tile([64, 2], F32, name="arb2i")
            arb2o = dp.tile([64, 2], F32, name="arb2o")

            RG = [list(range(NC))]

            def build_table(l, src_sb):
                """src_sb [64, NPC] + eb_l -> transpose -> AG -> tabs[l%2]."""
                nc.vector.tensor_scalar(out=zt_work[:], in0=src_sb[:],
                                        scalar1=colp[0:64, 6 * l:6 * l + 1],
                                        scalar2=None, op0=mybir.AluOpType.add)
                bnc = agb[l % 2]
                for ti, (o, w) in enumerate(TT):
                    pt = paux.tile([128, 64], F32, name=f"ptr{l}_{ti}",
                                   tag="ptr")
                    nc.tensor.transpose(pt[0:w, :], zt_work[:, o:o + w],
                                        identf[:])
                    ts_ = smp.tile([128, 64], F32, name=f"trs{l}_{ti}",
                                   tag="trs")
                    nc.scalar.copy(ts_[0:w, :], pt[0:w, :])
                    nc.sync.dma_start(bnc[o:o + w, :], ts_[0:w, :])
                nc.gpsimd.collective_compute(
                    "AllGather", mybir.AluOpType.bypass, replica_groups=RG,
                    ins=[bnc[:].opt()], outs=[tabs[l][:].opt()])

            def batchnorm_cols(stats, bnc, g_col, b_col, nfrac):
                """stats [P,2] (sum, sumsq) -> sc=bnc[:,6], bi=bnc[:,5]."""
                P = stats.shape[0]
                nc.vector.tensor_scalar(out=bnc[:, 0:2], in0=stats[:, 0:2],
                                        scalar1=nfrac, scalar2=None,
                                        op0=mybir.AluOpType.mult)
                nc.vector.tensor_mul(bnc[:, 2:3], bnc[:, 0:1], bnc[:, 0:1])
                nc.vector.tensor_sub(bnc[:, 3:4], bnc[:, 1:2], bnc[:, 2:3])
                nc.scalar.activation(bnc[:, 4:5], bnc[:, 3:4],
                                     mybir.ActivationFunctionType.Sqrt,
                                     bias=epsb[0:P, 0:1])
                nc.vector.reciprocal(bnc[:, 5:6], bnc[:, 4:5])
                nc.vector.tensor_mul(bnc[:, 6:7], bnc[:, 5:6], g_col)
                nc.vector.tensor_mul(bnc[:, 7:8], bnc[:, 0:1], bnc[:, 6:7])
                nc.vector.tensor_sub(bnc[:, 5:6], b_col, bnc[:, 7:8])

            ar_ct = [0]

            def allreduce_stats(stats_ap, bi, bo, P):
                nc.sync.dma_start(bi[:], stats_ap)
                nc.gpsimd.collective_compute(
                    "AllReduce", mybir.AluOpType.add, replica_groups=RG,
                    ins=[bi[:].opt()], outs=[bo[:].opt()])
                ar_ct[0] += 1
                sb_t = smp.tile([P, 2], F32, name=f"ars{ar_ct[0]}",
                                tag=f"ars{P}")
                nc.sync.dma_start(sb_t[:], bo[:])
                return sb_t

            # ---------- prologue: x0 = lin0^T @ xT + b ----------
            for i, (o, w) in enumerate(MT):
                pm = pmlp.tile([64, 512], F32, name=f"px0_{i}", tag="pmlp")
                nc.tensor.matmul(pm[:, 0:w], l0w[:], xT[:, o:o + w])
                nc.scalar.activation(z_work[:, o:o + w], pm[0:64, 0:w],
                                     mybir.ActivationFunctionType.Identity,
                                     bias=colp[0:64, L * 6:L * 6 + 1])
            build_table(0, z_work)

            # ---------- layers ----------
            for l in range(L):
                t_l = tvals[l]
                tab = tabs[l]
                nc.vector.memset(epsb[:, 1:2], t_l * EPS)

                for s in range(SLOTS):
                    stag = stp.tile([128, NCH_S, C], F32, name=f"st{l}_{s}",
                                    tag="stag")
                    qp = qpp.tile([128, NCH_S, 128], BF16, name=f"qp{l}_{s}",
                                  tag="qp")
                    ch0 = s * NCH_S
                    icol0 = s * NCH_S * 8
                    for j, (c0, nch) in enumerate(splits):
                        L_idx = nch * 128
                        nc.gpsimd.dma_gather(
                            stag[:, c0:c0 + nch, :], tab[bases[j]:N, :],
                            idx_all[:, icol0 + c0 * 8:icol0 + (c0 + nch) * 8],
                            L_idx, L_idx, C)
                    # ef = ea (bcast over C) * ew (bcast over chunks)
                    ea_b = ea_all[:, ch0:ch0 + NCH_S].unsqueeze(2).to_broadcast(
                        [128, NCH_S, C])
                    ew_b = ewr[:, l * C:(l + 1) * C].unsqueeze(1).to_broadcast(
                        [128, NCH_S, C])
                    nc.vector.tensor_tensor(out=qp[:, :, 0:C], in0=ea_b,
                                            in1=ew_b, op=mybir.AluOpType.mult)
                    nc.vector.tensor_add(stag[:], stag[:], qp[:, :, 0:C])
                    nc.scalar.activation(stag[:], stag[:],
                                         mybir.ActivationFunctionType.Relu)
                    nc.scalar.activation(qp[:, :, C:128], stag[:],
                                         mybir.ActivationFunctionType.Exp,
                                         bias=epsb[:, 1:2], scale=t_l)
                    nc.vector.scalar_tensor_tensor(
                        out=qp[:, :, 0:C], in0=stag[:], scalar=EPS,
                        in1=qp[:, :, C:128],
                        op0=mybir.AluOpType.add, op1=mybir.AluOpType.mult)
                    # per-chunk one-hot scatter matmuls
                    ps = pagg.tile([128, SW], F32, name=f"pa{l}_{s}",
                                   tag="pagg")
                    for kk in range(NCH_S):
                        g = ch0 + kk
                        St = sp.tile([128, SW], BF16, name=f"S{l}_{s}_{kk}",
                                     tag="S")
                        dl_b = dl_all[:, g:g + 1].to_broadcast([128, SW])
                        nc.vector.tensor_tensor(out=St[:], in0=iota[:],
                                                in1=dl_b,
                                                op=mybir.AluOpType.is_equal)
                        nc.tensor.matmul(ps[:], qp[:, kk, :], St[:],
                                         start=(kk == 0),
                                         stop=(kk == NCH_S - 1))
                    # close slot: out = num/den + z
                    aggs = smp.tile([128, SW], F32, name=f"ag{l}_{s}",
                                    tag="aggs")
                    nc.scalar.copy(aggs[:], ps[:])
                    dps = paux.tile([64, SW], F32, name=f"dp{l}_{s}",
                                    tag="dps")
                    nc.tensor.matmul(dps[:], shift[:], aggs[:])
                    den = smp.tile([64, SN], F32, name=f"dn{l}_{s}", tag="den")
                    nc.vector.tensor_scalar(out=den[:], in0=dps[0:64, 0:SN],
                                            scalar1=1e-30, scalar2=None,
                                            op0=mybir.AluOpType.max)
                    rec = smp.tile([64, SN], F32, name=f"rc{l}_{s}", tag="rec")
                    nc.vector.reciprocal(rec[:], den[:])
                    o0 = s * SN
                    nc.vector.tensor_mul(agg_work[:, o0:o0 + SN],
                                         aggs[0:64, 0:SN], rec[:])
                    nc.vector.tensor_add(agg_work[:, o0:o0 + SN],
                                         agg_work[:, o0:o0 + SN],
                                         z_work[:, o0:o0 + SN])

                # ---------- node phase ----------
                for i, (o, w) in enumerate(MT):
                    pm = pmlp.tile([128, 512], F32, name=f"pm1_{l}_{i}",
                                   tag="pmlp")
                    nc.tensor.matmul(pm[:, 0:w], w1[:, l * CH:(l + 1) * CH],
                                     agg_work[:, o:o + w])
                    nc.scalar.activation(h1_work[:, o:o + w], pm[:, 0:w],
                                         mybir.ActivationFunctionType.Copy,
                                         accum_out=stats1[:, i:i + 1])
                    nc.scalar.activation(dump[:, 0:w], pm[:, 0:w],
                                         mybir.ActivationFunctionType.Square,
                                         accum_out=stats1[:, 16 + i:17 + i])
                nc.vector.tensor_reduce(stats1[:, 10:11], stats1[:, 0:10],
                                        mybir.AxisListType.X,
                                        mybir.AluOpType.add)
                nc.vector.tensor_reduce(stats1[:, 26:27], stats1[:, 16:26],
                                        mybir.AxisListType.X,
                                        mybir.AluOpType.add)
                nc.vector.tensor_copy(stats1[:, 11:12], stats1[:, 26:27])
                st1 = allreduce_stats(stats1[:, 10:12], arb1i, arb1o, 128)
                batchnorm_cols(st1, bnc1, colp[:, 6 * l + 1:6 * l + 2],
                               colp[:, 6 * l + 2:6 * l + 3], 1.0 / N)
                nc.scalar.activation(h1_work[:], h1_work[:],
                                     mybir.ActivationFunctionType.Relu,
                                     bias=bnc1[:, 5:6], scale=bnc1[:, 6:7])
                for i, (o, w) in enumerate(MT):
                    pm = pmlp.tile([64, 512], F32, name=f"pm2_{l}_{i}",
                                   tag="pmlp")
                    nc.tensor.matmul(pm[:, 0:w], w2[:, l * C:(l + 1) * C],
                                     h1_work[:, o:o + w])
                    if l == 0:
                        nc.vector.tensor_scalar(
                            out=h_work[:, o:o + w], in0=pm[0:64, 0:w],
                            scalar1=colp[0:64, 6 * l + 3:6 * l + 4],
                            scalar2=None, op0=mybir.AluOpType.add)
                    else:
                        nc.vector.scalar_tensor_tensor(
                            out=h_work[:, o:o + w], in0=pm[0:64, 0:w],
                            scalar=colp[0:64, 6 * l + 3:6 * l + 4],
                            in1=h_work[:, o:o + w],
                            op0=mybir.AluOpType.add, op1=mybir.AluOpType.add)
                # BN2 stats (pre-norm for next layer / final)
                for i, (o, w) in enumerate(MT):
                    nc.vector.tensor_reduce(stats2[:, i:i + 1],
                                            h_work[:, o:o + w],
                                            mybir.AxisListType.X,
                                            mybir.AluOpType.add)
                    nc.scalar.activation(dump[0:64, 0:w], h_work[:, o:o + w],
                                         mybir.ActivationFunctionType.Square,
                                         accum_out=stats2[:, 16 + i:17 + i])
                nc.vector.tensor_reduce(stats2[:, 10:11], stats2[:, 0:10],
                                        mybir.AxisListType.X,
                                        mybir.AluOpType.add)
                nc.vector.tensor_reduce(stats2[:, 26:27], stats2[:, 16:26],
                                        mybir.AxisListType.X,
                                        mybir.AluOpType.add)
                nc.vector.tensor_copy(stats2[:, 11:12], stats2[:, 26:27])
                st2 = allreduce_stats(stats2[:, 10:12], arb2i, arb2o, 64)
                batchnorm_cols(st2, bnc2, colp[0:64, 6 * l + 4:6 * l + 5],
                               colp[0:64, 6 * l + 5:6 * l + 6], 1.0 / N)
                nc.scalar.activation(z_work[:], h_work[:],
                                     mybir.ActivationFunctionType.Lrelu,
                                     bias=bnc2[:, 5:6], scale=bnc2[:, 6:7],
                                     alpha=SLOPE)
                if l < L - 1:
                    build_table(l + 1, z_work)
                else:
                    for ti, (o, w) in enumerate(TT):
                        pt = paux.tile([128, 64], F32, name=f"pto_{ti}",
                                       tag="ptr")
                        nc.tensor.transpose(pt[0:w, :], z_work[:, o:o + w],
                                            identf[:])
                        ts_ = smp.tile([128, 64], F32, name=f"tso_{ti}",
                                       tag="trs")
                        nc.scalar.copy(ts_[0:w, :], pt[0:w, :])
                        nc.sync.dma_start(d_out[o:o + w, :], ts_[0:w, :])

    nc.compile()
    return nc


def kernel(**inputs):
    _shim_ntff_hook()
    per_core, shared, meta = _host_prep(inputs)
    nc = _build_program(shared, meta)
    in_maps = []
    for c in range(NC):
        m = dict(shared)
        m.update(per_core[c])
        in_maps.append(m)
    trace = os.environ.get("BASS_KERNEL_TRACE", "0") == "1"
    res = bass_utils.run_bass_kernel_spmd(nc, in_maps,
                                          core_ids=list(range(NC)),
                                          trace=trace)
    out = np.concatenate([res.results[c]["out"] for c in range(NC)], axis=0)
    final = out[meta["new_id"]]
    if trace:
        kernel.last_exec_ns = res.exec_time_ns
        kernel.last_profile = res
    return final.astype(np.float32)


if __name__ == "__main__":
    import reference as R
    import jax
    with jax.default_device(jax.devices("cpu")[0]):
        import jax.numpy as jnp
        inputs = {k: np.asarray(v) for k, v in R.setup_inputs().items()}
        expected = np.asarray(R.reference(
            **{k: jnp.asarray(v) for k, v in inputs.items()}))
    actual = kernel(**inputs)
    rel = np.linalg.norm(actual - expected) / np.linalg.norm(expected)
    print("Relative error:", rel)


